# revision 2
# baseline (speedup 1.0000x reference)
"""Raw-bass fp8 (DoubleRow) Trainium2 kernel, v4 — single-ring input stream.

Math as kernel2/3 (hi/lo fp8 split, x16 ws scale, host 1/(16*M) epilogue).

v4 vs v3 (from the v3 trace):
- A DMA transfer's completion semaphore is the LAST of 16 per-engine +1
  increments; cross-ring packet contention stretches that tail by ~2us, and
  the two rings' descriptor preps serialize (~1.5-2us each). So ALL inputs
  ride ONE ring (sync) as three seamless transfers of one packed fp8 tensor,
  ordered by need: T1 = ws subs 0:4 | wlr | vh subs 0:4 (phase A set),
  T2 = ws subs 4:8 | vh subs 4:8, T3 = vl. Outputs ride ring B (scalar,
  idle otherwise) plus ring A for o1/o3b.
- The +A broadcast add for chunks 8..14 moved to the PE: one 448-col bf16
  matmul per block, lhsT=TRI, rhs = rs broadcast (stride-0) accumulating
  into psum[512:960). ACT (scalar) then drains psum[512:1024) -> o_sb bf16
  (also covers chunk15 = A, killing the c15 copy). DVE keeps rs/a copies and
  the [0:512) broadcast adds. Tail engines: PE / ACT / DVE balanced.
"""

import os
import sys
import types

import numpy as np
import ml_dtypes

if "/opt/trn_rl_repo" not in sys.path:
    sys.path.insert(0, "/opt/trn_rl_repo")

try:
    import antenv.axon_hooks  # noqa: F401
except ImportError:
    _m = types.ModuleType("antenv.axon_hooks")

    def _get_hook():
        try:
            from trn_agent_boot.trn_boot import _ntff_profile_via_ctypes

            return _ntff_profile_via_ctypes("/opt/axon/libaxon_pjrt.so")
        except Exception:
            return None

    _m.get_axon_ntff_profile_hook = _get_hook
    sys.modules["antenv.axon_hooks"] = _m

import concourse.bacc as bacc
import concourse.mybir as mybir
from concourse.bass_utils import run_bass_kernel_spmd

B, S, N = 2, 2048, 1024
H, HD = 16, 64
NB = B * H
N_CORES = 8
PER_CORE = NB // N_CORES  # 4
M_SUM = float(B * H * S * (S - 1) // 2)
SC = 16.0

F32 = mybir.dt.float32
BF16 = mybir.dt.bfloat16
FP8 = mybir.dt.float8e4
NP_BF16 = np.dtype(ml_dtypes.bfloat16)
NP_FP8 = np.dtype(ml_dtypes.float8_e4m3)
DR = mybir.MatmulPerfMode.DoubleRow

N_DUMMY = 9

# packed input column offsets (bytes per partition), ordered by PE need
# (one transfer per k-tile set; vl arrives BEFORE the t3 set so the vl chains
# can run between the t2 and t3 mains with zero stalls):
# TA: ws s01 | wlr | vh s01   TB: ws s23 | vh s23   TC1: ws s45 | vh s45
# TD: vl                      TC2: ws s67 | vh s67
O_WS0, O_WLR, O_VH0 = 0, 2048, 2560
O_WS1, O_VH1 = 3584, 5632
O_WS2T2, O_VH2T2 = 6656, 8704
O_VL = 9728
O_WS2T3, O_VH2T3 = 13824, 15872
O_END = 16896

_compiled = None
_last_exec_time_ns = None
_last_results = None

# PE sem values: R_j after mains(j, t3) (the last R-region writer); AB_j after
# the A matmuls for block j
P_R0, P_R1, P_AB0, P_R2, P_AB1, P_R3, P_AB2, P_AB3 = 1, 2, 3, 4, 5, 6, 7, 8
# DVE sem values (program order)
V_RS0, V_RS1, V_A0, V_ADDA0, V_RS2, V_A1, V_ADDA1, V_RS3 = 1, 2, 3, 4, 5, 6, 7, 8
V_A2, V_ADDA2, V_A3, V_ADDA3 = 9, 10, 11, 12
# ACT sem values
C_D0, C_D1, C_D2, C_D3 = 1, 2, 3, 4

DMA_NAMES = ["ta", "tb", "tc1", "td", "tc2", "o0", "o1", "o2", "o3a", "o3b"]


def _build_nc():
    nc = bacc.Bacc(
        "TRN2", target_bir_lowering=False, debug=False, enable_asserts=False
    )
    din_d = nc.dram_tensor("din", [128, O_END], FP8, kind="ExternalInput").ap()
    out_d = nc.dram_tensor("out", [128, PER_CORE, 1024], BF16, kind="ExternalOutput").ap()

    in_sb = nc.alloc_sbuf_tensor("in_sb", [128, O_END], FP8).ap()
    tri_sb = nc.alloc_sbuf_tensor("tri_sb", [128, 128], BF16).ap()
    warm_sb = nc.alloc_sbuf_tensor("warm_sb", [128, 640], FP8).ap()
    rs_sb = [
        nc.alloc_sbuf_tensor(f"rs_sb{j}", [128, HD], BF16).ap()
        for j in range(PER_CORE)
    ]
    a_sb = [
        nc.alloc_sbuf_tensor(f"a_sb{j}", [128, HD], F32).ap() for j in range(PER_CORE)
    ]
    o_sb = nc.alloc_sbuf_tensor("o_sb", [128, PER_CORE, 1024], BF16).ap()

    p = [nc.alloc_psum_tensor(f"p{j}", [128, 1024], F32).ap() for j in range(PER_CORE)]

    sems = {
        k: nc.alloc_semaphore(f"sem_{k}")
        for k in ["PE", "DVE", "ACT", "tri"] + DMA_NAMES
    }
    sem_nums = [s.num for s in sems.values()]
    sem_range = range(min(sem_nums), max(sem_nums) + 1)
    assert max(sem_nums) - min(sem_nums) == len(sem_nums) - 1

    ws_base = {0: O_WS0, 1: O_WS1, 2: O_WS2T2, 3: O_WS2T3}
    vh_base = {0: O_VH0, 1: O_VH1, 2: O_VH2T2, 3: O_VH2T3}

    def vh_tile(j, t):
        o = vh_base[t]
        return in_sb[:, o : o + 1024].rearrange("p (s j r) -> p s j r", s=2, j=4)[
            :, :, j, :
        ]

    def wlr_tile(t):
        o = O_WLR + 128 * t
        return in_sb[:, o : o + 128].rearrange("p (two m) -> p two m", two=2)

    def vl_tile(j, t):
        o = O_VL + 1024 * j + 256 * t
        return in_sb[:, o : o + 256].rearrange("p (two m) -> p two m", two=2)

    def ws_tile(t, lo, hi):
        o = ws_base[t]
        return in_sb[:, o : o + 2048].rearrange("p (s c) -> p s c", s=2)[:, :, lo:hi]

    # ---- pre-Block: DMA issues (all inputs on ring A), warmup, tri ---------
    for name, lo, hi in [
        ("ta", O_WS0, O_WS1),
        ("tb", O_WS1, O_WS2T2),
        ("tc1", O_WS2T2, O_VL),
        ("td", O_VL, O_WS2T3),
        ("tc2", O_WS2T3, O_END),
    ]:
        nc.sync.dma_start(in_sb[:, lo:hi], din_d[:, lo:hi]).then_inc(sems[name], 16)

    for _i in range(N_DUMMY):
        nc.tensor.matmul(
            p[3][:, 512:1024],
            warm_sb[:, 0:128],
            warm_sb[:, 128:640],
            start=True,
            stop=True,
            skip_group_check=True,
        )

    nc.gpsimd.memset(tri_sb[:], 1.0)
    nc.gpsimd.affine_select(
        tri_sb[:],
        tri_sb[:],
        pattern=[[-1, 128]],
        compare_op=mybir.AluOpType.is_gt,
        fill=0.0,
        base=0,
        channel_multiplier=1,
    ).then_inc(sems["tri"], 1)

    with nc.Block() as block:

        @block.tensor
        def _(tensor):
            def mains(j, t, first=False, last=False):
                nc.tensor.matmul(
                    p[j][:, 0:512],
                    vh_tile(j, t),
                    ws_tile(t, 0, 512),
                    start=first,
                    stop=last,
                    perf_mode=DR,
                    skip_group_check=True,
                )
                if not first:
                    nc.tensor.matmul(
                        p[j][:, 960:1024],
                        vh_tile(j, t),
                        wlr_tile(t),
                        start=False,
                        stop=False,
                        perf_mode=DR,
                        skip_group_check=True,
                    )
                nc.tensor.matmul(
                    p[j][:, 512:1024],
                    vh_tile(j, t),
                    ws_tile(t, 512, 1024),
                    start=first,
                    stop=False,
                    perf_mode=DR,
                    skip_group_check=True,
                )
                if first:
                    nc.tensor.matmul(
                        p[j][:, 960:1024],
                        vh_tile(j, t),
                        wlr_tile(t),
                        start=False,
                        stop=False,
                        perf_mode=DR,
                        skip_group_check=True,
                    )

            def vl_chain(j, tmax=4):
                for t in range(tmax):
                    nc.tensor.matmul(
                        p[j][:, 960:1024],
                        vl_tile(j, t),
                        ws_tile(t, 960, 1024),
                        start=False,
                        stop=False,
                        perf_mode=DR,
                        skip_group_check=True,
                    )

            def a_mms(j, dve_val):
                # Broadcast-add A to chunks 8..14 (stride-0 rhs, accumulate)
                # FIRST, then A_j into [960:1024): the start=True write marks a
                # pending-zero zone wider than 64 cols, so it must come last.
                tensor.wait_ge(sems["DVE"], dve_val)
                nc.tensor.matmul(
                    p[j][:, 512:960].rearrange("p (g d) -> p g d", d=HD),
                    tri_sb[:],
                    rs_sb[j][:].unsqueeze(1).broadcast_to([128, 7, HD]),
                    start=False,
                    stop=True,
                    skip_group_check=True,
                )
                nc.tensor.matmul(
                    p[j][:, 960:1024],
                    tri_sb[:],
                    rs_sb[j][:],
                    start=True,
                    stop=True,
                    skip_group_check=True,
                ).then_inc(sems["PE"], 1)

            def mains_r(j):
                # t3 mains + the t3 vl matmul (its whr cols arrive with TC2);
                # the trailing mmB is the last R-region writer
                nc.tensor.matmul(
                    p[j][:, 0:512],
                    vh_tile(j, 3),
                    ws_tile(3, 0, 512),
                    start=False,
                    stop=True,
                    perf_mode=DR,
                    skip_group_check=True,
                )
                nc.tensor.matmul(
                    p[j][:, 960:1024],
                    vh_tile(j, 3),
                    wlr_tile(3),
                    start=False,
                    stop=False,
                    perf_mode=DR,
                    skip_group_check=True,
                )
                nc.tensor.matmul(
                    p[j][:, 960:1024],
                    vl_tile(j, 3),
                    ws_tile(3, 960, 1024),
                    start=False,
                    stop=False,
                    perf_mode=DR,
                    skip_group_check=True,
                )
                nc.tensor.matmul(
                    p[j][:, 512:1024],
                    vh_tile(j, 3),
                    ws_tile(3, 512, 1024),
                    start=False,
                    stop=True,
                    perf_mode=DR,
                    skip_group_check=True,
                ).then_inc(sems["PE"], 1)

            tensor.wait_ge(sems["ta"], 16)
            for j in range(PER_CORE):
                mains(j, 0, first=True)
            tensor.wait_ge(sems["tb"], 16)
            for j in range(PER_CORE):
                mains(j, 1)
            tensor.wait_ge(sems["tc1"], 16)
            for j in range(PER_CORE):
                mains(j, 2)
            tensor.wait_ge(sems["td"], 16)
            for j in range(PER_CORE):
                vl_chain(j, tmax=3)
            tensor.wait_ge(sems["tc2"], 16)
            tensor.wait_ge(sems["tri"], 1)
            mains_r(0)  # PE=1 (R0)
            mains_r(1)  # PE=2 (R1)
            a_mms(0, V_RS0)  # PE=3 (AB0)
            mains_r(2)  # PE=4 (R2)
            a_mms(1, V_RS1)  # PE=5 (AB1)
            mains_r(3)  # PE=6 (R3)
            a_mms(2, V_RS2)  # PE=7 (AB2)
            a_mms(3, V_RS3)  # PE=8 (AB3)

        @block.scalar
        def _(scalar):
            def drain(j, pe_val):
                scalar.wait_ge(sems["PE"], pe_val)
                nc.scalar.copy(o_sb[:, j, 512:1024], p[j][:, 512:1024]).then_inc(
                    sems["ACT"], 1
                )

            drain(0, P_AB0)  # ACT=1
            scalar.wait_ge(sems["DVE"], V_ADDA0)
            scalar.dma_start(out_d[:, 0, :], o_sb[:, 0, :]).then_inc(sems["o0"], 16)
            drain(1, P_AB1)  # ACT=2
            drain(2, P_AB2)  # ACT=3
            scalar.wait_ge(sems["DVE"], V_ADDA2)
            scalar.dma_start(out_d[:, 2, :], o_sb[:, 2, :]).then_inc(sems["o2"], 16)
            drain(3, P_AB3)  # ACT=4
            # o3b right after our own drain3 — no cross-engine hop
            scalar.dma_start(out_d[:, 3, 512:1024], o_sb[:, 3, 512:1024]).then_inc(
                sems["o3b"], 16
            )

        @block.sync
        def _(sync):
            sync.wait_ge(sems["DVE"], V_ADDA1)
            sync.wait_ge(sems["ACT"], C_D1)
            sync.dma_start(out_d[:, 1, :], o_sb[:, 1, :]).then_inc(sems["o1"], 16)
            sync.wait_ge(sems["DVE"], V_ADDA3)
            sync.dma_start(out_d[:, 3, 0:512], o_sb[:, 3, 0:512]).then_inc(
                sems["o3a"], 16
            )

        @block.vector
        def _(vector):
            def rs_copy(j, pe_val):
                vector.wait_ge(sems["PE"], pe_val)
                nc.vector.tensor_copy(rs_sb[j][:], p[j][:, 960:1024]).then_inc(
                    sems["DVE"], 1
                )

            def a_copy(j, pe_val):
                vector.wait_ge(sems["PE"], pe_val)
                nc.vector.tensor_copy(a_sb[j][:], p[j][:, 960:1024]).then_inc(
                    sems["DVE"], 1
                )

            def addA(j):
                nc.vector.tensor_add(
                    o_sb[:, j, 0:512].rearrange("p (g d) -> p g d", d=HD),
                    p[j][:, 0:512].rearrange("p (g d) -> p g d", d=HD),
                    a_sb[j][:].unsqueeze(1).broadcast_to([128, 8, HD]),
                ).then_inc(sems["DVE"], 1)

            rs_copy(0, P_R0)  # DVE=1
            rs_copy(1, P_R1)  # DVE=2
            a_copy(0, P_AB0)  # DVE=3
            addA(0)  # DVE=4
            rs_copy(2, P_R2)  # DVE=5
            a_copy(1, P_AB1)  # DVE=6
            addA(1)  # DVE=7
            rs_copy(3, P_R3)  # DVE=8
            a_copy(2, P_AB2)  # DVE=9
            addA(2)  # DVE=10
            a_copy(3, P_AB3)  # DVE=11
            addA(3)  # DVE=12

        @block.gpsimd
        def _(gpsimd):
            for name in ("o0", "o1", "o2", "o3a", "o3b"):
                gpsimd.wait_ge(sems[name], 16)

    nc.gpsimd.sem_clear(sem_range)

    nc.compile()
    return nc


def _q8(x):
    return np.clip(x, -240.0, 240.0).astype(NP_FP8)


def _host_prep(v, WV):
    WVr = WV.astype(np.float64).reshape(N, 16, HD)
    rev = np.flip(np.cumsum(np.flip(WVr, axis=1), axis=1), axis=1)
    WVS = rev - WVr
    WVR = rev[:, 0, :]
    ws_cols = np.empty((N, 1024), np.float64)
    ws_cols[:, 0:512] = WVS[:, :8, :].reshape(N, 512)
    ws_cols[:, 512:960] = WVS[:, 8:15, :].reshape(N, 448)
    ws_cols[:, 960:1024] = WVR
    ws_cols *= SC
    ws_h = _q8(ws_cols)
    wlr = _q8(ws_cols[:, 960:1024] - ws_h[:, 960:1024].astype(np.float64))
    ws_dev = ws_h.reshape(8, 128, 1024).transpose(1, 0, 2)  # [p, s, c]
    wlr_dev = (
        wlr.reshape(4, 2, 128, HD).transpose(2, 0, 1, 3).reshape(128, 512)
    )  # [p, 128t+64i+d]

    vt_all = np.empty((NB, 128, 8, 128), dtype=np.float32)
    for g in range(NB):
        b, h = divmod(g, H)
        vb = v[b, 128 * h : 128 * (h + 1), :]
        vt_all[g] = vb.T.reshape(8, 128, 128).transpose(1, 0, 2)
    vh8 = _q8(vt_all)
    vl8 = _q8(vt_all - vh8.astype(np.float32))
    return ws_dev, wlr_dev, vh8, vl8


def kernel(q, k, v, WQ, WK, WV):
    global _compiled, _last_exec_time_ns, _last_results
    v = np.ascontiguousarray(np.asarray(v, dtype=np.float32))
    WV = np.ascontiguousarray(np.asarray(WV, dtype=np.float32))
    ws_dev, wlr_dev, vh8, vl8 = _host_prep(v, WV)

    if _compiled is None:
        _compiled = _build_nc()
    nc = _compiled

    in_maps = []
    for c in range(N_CORES):
        vh_c = vh8[PER_CORE * c : PER_CORE * (c + 1)]  # [j, p, s, r]
        vl_c = vl8[PER_CORE * c : PER_CORE * (c + 1)]
        din = np.empty((128, O_END), dtype=NP_FP8)

        def pack_vh(sl):
            return vh_c[:, :, sl, :].transpose(1, 2, 0, 3).reshape(128, 1024)

        din[:, O_WS0:O_WLR] = ws_dev[:, 0:2, :].reshape(128, 2048)
        din[:, O_WLR:O_VH0] = wlr_dev
        din[:, O_VH0:O_WS1] = pack_vh(slice(0, 2))
        din[:, O_WS1:O_VH1] = ws_dev[:, 2:4, :].reshape(128, 2048)
        din[:, O_VH1:O_WS2T2] = pack_vh(slice(2, 4))
        din[:, O_WS2T2:O_VH2T2] = ws_dev[:, 4:6, :].reshape(128, 2048)
        din[:, O_VH2T2:O_VL] = pack_vh(slice(4, 6))
        din[:, O_VL:O_WS2T3] = vl_c.transpose(1, 0, 2, 3).reshape(128, 4096)
        din[:, O_WS2T3:O_VH2T3] = ws_dev[:, 6:8, :].reshape(128, 2048)
        din[:, O_VH2T3:O_END] = pack_vh(slice(6, 8))
        in_maps.append({"din": np.ascontiguousarray(din)})

    res = run_bass_kernel_spmd(
        nc,
        in_maps,
        core_ids=list(range(N_CORES)),
        tmpdir=os.environ.get("BASS_KERNEL_TRACE_DIR") or None,
    )
    _last_exec_time_ns = res.exec_time_ns
    _last_results = res

    inv = 1.0 / (SC * M_SUM)
    out = np.empty((B, S, N), dtype=np.float32)
    for c in range(N_CORES):
        oh = res.results[c]["out"]
        ohf = oh.astype(np.float32) * inv
        for j in range(PER_CORE):
            g = PER_CORE * c + j
            b, h = divmod(g, H)
            out[b, :, HD * h : HD * (h + 1)] = ohf[:, j, :].reshape(S, HD)
    return out
